# revision 9
# baseline (speedup 1.0000x reference)
"""Trainium2 Bass kernel for nn_BinaryTreeLogicNet.

Computes, for x [B=65536, N=1024] f32:
  10-level balanced binary tree reduction over the feature dim, where each
  merge(a_node, b_node) with per-merge params (w0, w1, bias) is:
      a = |a_node * w0| + EPS
      b = |b_node * w1| + EPS
      lam = sigmoid(bias)
      out = a^lam * b^(1-lam) + (1-lam) * max(a, b)
  followed by y = sigmoid(root * fc_w + fc_b)  -> [B, 1]

Sharding: pure data parallel over the batch dim across 8 NeuronCores
(8192 rows/core); weights/biases are replicated (preprocessed on host).

Device algorithm (per 128-row batch tile, batch on SBUF partitions):
  ax    = |x|                       (tensor_scalar abs_max)
  per level l (n inputs, m = n/2 outputs):
    prod = cur * W_l                (W_l = interleaved |w0|,|w1| broadcast)
    lab  = Ln(prod + EPS)           (ScalarE, bias=EPS)
    s    = lab_even - lab_odd
    t    = s * lam + lab_odd        => t = lam*la + (1-lam)*lb
    p    = Exp(t)                   = a^lam * b^(1-lam)
    mx   = max(prod_even, prod_odd) = max(a,b) - EPS
    q    = (mx + EPS) * om          = (1-lam) * max(a,b)
    cur' = p + q
  head: e = Exp(-(fc_w*root + fc_b)); y = 1/(1+e)
"""

import sys

if "/opt/trn_rl_repo" not in sys.path:
    sys.path.insert(0, "/opt/trn_rl_repo")

import numpy as np

import concourse.bacc as bacc
import concourse.bass as bass
import concourse.mybir as mybir
import concourse.tile as tile
from concourse.bass_utils import run_bass_kernel_spmd

F32 = mybir.dt.float32
ALU = mybir.AluOpType
ACTF = mybir.ActivationFunctionType

N = 1024
B = 65536
NCORES = 8
ROWS_PER_CORE = B // NCORES
EPS = 1e-6


def _level_consts(weights: np.ndarray, biases: np.ndarray):
    """Host preprocessing of the (N-1) merge params into per-level flat arrays.

    Returns (wflat [2046], lamflat [1023], omflat [1023]) where wflat holds,
    per level, |w0|,|w1| interleaved in node order.
    """
    wflat = np.empty(2 * (N - 1), dtype=np.float32)
    lamflat = np.empty(N - 1, dtype=np.float32)
    off = 0  # param offset
    woff = 0
    n = N
    while n > 1:
        m = n // 2
        w = weights[off : off + m].astype(np.float64)
        lam = 1.0 / (1.0 + np.exp(-biases[off : off + m].astype(np.float64)))
        wseg = np.empty(2 * m, dtype=np.float32)
        wseg[0::2] = np.abs(w[:, 0]).astype(np.float32)
        wseg[1::2] = np.abs(w[:, 1]).astype(np.float32)
        wflat[woff : woff + 2 * m] = wseg
        lamflat[off : off + m] = lam.astype(np.float32)
        off += m
        woff += 2 * m
        n = m
    omflat = (1.0 - lamflat.astype(np.float64)).astype(np.float32)
    return wflat, lamflat, omflat


def build_kernel(rows: int, fc_w: float, fc_b: float) -> bass.Bass:
    """Build the single-core Bass module (SPMD across cores)."""
    nc = bacc.Bacc("TRN2", target_bir_lowering=False)
    ntiles = rows // 128
    x_d = nc.declare_dram_parameter("x", [rows, N], F32, isOutput=False)
    wt_d = nc.declare_dram_parameter("wt", [128, 2 * (N - 1)], F32, isOutput=False)
    lam_d = nc.declare_dram_parameter("lam", [128, N - 1], F32, isOutput=False)
    om_d = nc.declare_dram_parameter("om", [128, N - 1], F32, isOutput=False)
    # y[p, t] = output of batch row t*128 + p (host transposes back)
    y_d = nc.declare_dram_parameter("y", [128, ntiles], F32, isOutput=True)

    with tile.TileContext(nc) as tc:
        with (
            tc.tile_pool(name="consts", bufs=1) as cpool,
            tc.tile_pool(name="work", bufs=2) as pool,
        ):
            eps_b = cpool.tile([128, 1], F32)
            nc.gpsimd.memset(eps_b[:], EPS)
            head_b = cpool.tile([128, 1], F32)
            nc.gpsimd.memset(head_b[:], -float(fc_b))
            wt = cpool.tile([128, 2 * (N - 1)], F32)
            nc.sync.dma_start(out=wt[:], in_=wt_d[:])
            lam = cpool.tile([128, N - 1], F32)
            nc.sync.dma_start(out=lam[:], in_=lam_d[:])
            om = cpool.tile([128, N - 1], F32)
            nc.sync.dma_start(out=om[:], in_=om_d[:])
            roots = cpool.tile([128, ntiles], F32)

            for ti in range(ntiles):
                xt = pool.tile([128, N], F32, tag="xt")
                nc.sync.dma_start(out=xt[:], in_=x_d[ti * 128 : (ti + 1) * 128, :])
                cur = pool.tile([128, N], F32, tag="cur0")
                nc.vector.tensor_scalar(
                    out=cur[:].bitcast(mybir.dt.uint32),
                    in0=xt[:].bitcast(mybir.dt.uint32),
                    scalar1=0x7FFFFFFF,
                    scalar2=None,
                    op0=ALU.bitwise_and,
                )
                woff = 0
                poff = 0
                for l in range(10):
                    n = N >> l
                    m = n >> 1
                    prod = pool.tile([128, N], F32, tag="prod")
                    nc.vector.tensor_mul(
                        out=prod[:, :n], in0=cur[:, :n], in1=wt[:, woff : woff + n]
                    )
                    lab = pool.tile([128, N], F32, tag="lab")
                    nc.scalar.activation(
                        out=lab[:, :n], in_=prod[:, :n], func=ACTF.Ln, bias=eps_b[:]
                    )
                    lab_e = lab[:, 0:n:2]
                    lab_o = lab[:, 1:n:2]
                    s = pool.tile([128, N // 2], F32, tag="s")
                    nc.vector.tensor_sub(out=s[:, :m], in0=lab_e, in1=lab_o)
                    t1 = pool.tile([128, N // 2], F32, tag="t1")
                    nc.vector.tensor_mul(
                        out=t1[:, :m], in0=s[:, :m], in1=lam[:, poff : poff + m]
                    )
                    t2 = pool.tile([128, N // 2], F32, tag="t2")
                    nc.vector.tensor_add(out=t2[:, :m], in0=t1[:, :m], in1=lab_o)
                    p = pool.tile([128, N // 2], F32, tag="p")
                    nc.scalar.activation(out=p[:, :m], in_=t2[:, :m], func=ACTF.Exp)
                    mx = pool.tile([128, N // 2], F32, tag="mx")
                    nc.vector.tensor_max(
                        out=mx[:, :m], in0=prod[:, 0:n:2], in1=prod[:, 1:n:2]
                    )
                    q = pool.tile([128, N // 2], F32, tag="q")
                    nc.vector.scalar_tensor_tensor(
                        out=q[:, :m],
                        in0=mx[:, :m],
                        scalar=EPS,
                        in1=om[:, poff : poff + m],
                        op0=ALU.add,
                        op1=ALU.mult,
                    )
                    newcur = pool.tile([128, N], F32, tag=f"cur{(l + 1) % 2}")
                    nc.vector.tensor_add(out=newcur[:, :m], in0=p[:, :m], in1=q[:, :m])
                    cur = newcur
                    woff += n
                    poff += m
                # head exp: roots[:, ti] = Exp(-(fc_w*root + fc_b))
                nc.scalar.activation(
                    out=roots[:, ti : ti + 1],
                    in_=cur[:, 0:1],
                    func=ACTF.Exp,
                    bias=head_b[:],
                    scale=-float(fc_w),
                )
            denom = cpool.tile([128, ntiles], F32)
            nc.vector.tensor_scalar(
                out=denom[:], in0=roots[:], scalar1=1.0, scalar2=None, op0=ALU.add
            )
            yt = cpool.tile([128, ntiles], F32)
            nc.vector.reciprocal(out=yt[:], in_=denom[:])
            nc.sync.dma_start(out=y_d[:], in_=yt[:])
    return nc


def _make_in_maps(x, weights, biases, rows, ncores):
    wflat, lamflat, omflat = _level_consts(weights, biases)
    wt = np.ascontiguousarray(np.broadcast_to(wflat, (128, 2 * (N - 1))))
    lm = np.ascontiguousarray(np.broadcast_to(lamflat, (128, N - 1)))
    om = np.ascontiguousarray(np.broadcast_to(omflat, (128, N - 1)))
    in_maps = []
    for c in range(ncores):
        shard = np.ascontiguousarray(x[c * rows : (c + 1) * rows])
        in_maps.append({"x": shard, "wt": wt, "lam": lm, "om": om})
    return in_maps


def run_spmd(x, weights, biases, fc_w, fc_b, **spmd_kwargs):
    """Build + run; returns (y_full, BassKernelResults)."""
    x = np.asarray(x)
    weights = np.asarray(weights)
    biases = np.asarray(biases)
    fc_w = float(np.asarray(fc_w))
    fc_b = float(np.asarray(fc_b))

    nc = build_kernel(ROWS_PER_CORE, fc_w, fc_b)
    in_maps = _make_in_maps(x, weights, biases, ROWS_PER_CORE, NCORES)
    if not nc.is_finalized():
        nc.finalize()
    res = run_bass_kernel_spmd(nc, in_maps, list(range(NCORES)), **spmd_kwargs)
    outs = []
    for c in range(NCORES):
        yc = res.results[c]["y"]  # [128, ntiles]
        outs.append(yc.T.reshape(-1, 1))  # row t*128+p = yc[p, t]
    y = np.ascontiguousarray(np.concatenate(outs, axis=0).astype(np.float32))
    return y, res


def kernel(x, weights, biases, fc_w, fc_b):
    y, _ = run_spmd(x, weights, biases, fc_w, fc_b)
    return y


if __name__ == "__main__":
    rng = np.random.default_rng(0)
    x = rng.standard_normal((B, N), dtype=np.float32)
    w = (rng.standard_normal((N - 1, 2)) * 0.1).astype(np.float32)
    b = (rng.random(N - 1) * 0.1).astype(np.float32)
    y = kernel(x, w, b, np.float32(0.5), np.float32(0.0))
    print(y.shape, y.dtype, y[:4, 0])
